# revision 1
# baseline (speedup 1.0000x reference)
"""Trainium2 Bass kernel for causal GQA attention (B=1, T=4096, D=2048,
H=16, Hkv=4, Dh=128, RoPE) sharded over 8 NeuronCores.

Sharding: tensor-parallel over heads — each core owns 2 q-heads and the
kv head they share (core c: q-heads {2c, 2c+1}, kv head c//2). Each core
computes its q/k/v projections, RoPE, causal attention and its partial
o_proj contribution y_c = O_c @ Wo_c; the host sums the 8 partials.

On-device dataflow (everything bf16 into the PE, f32 accumulation):
  xT tiles [c,t] -> Q^T/K^T/V^T [d,t] -> RoPE (DVE) -> S^T = K^T.T@Q^T
  per (j-tile 128, q-tile 512) -> exp on ACT (PSUM->SBUF bf16, fused
  1/sqrt(dh) scale) -> causal mask on diagonal blocks (DVE mul with
  precomputed mask) -> O_aug[q,129] += P^T.T @ [V | 1] (PE, PSUM
  accumulation; col 128 gives the softmax denominator) -> reciprocal +
  per-partition scale on ACT -> PE transpose -> O^T -> y = O^T.T @ Wo.
"""

import sys

sys.path.insert(0, "/opt/trn_rl_repo")

import math
from contextlib import ExitStack

import ml_dtypes
import numpy as np

import concourse.bass as bass
import concourse.tile as tile
from concourse import bacc, mybir
from concourse.bass_utils import run_bass_kernel_spmd
from concourse.masks import make_identity

BF16 = mybir.dt.bfloat16
F32 = mybir.dt.float32
NPBF16 = ml_dtypes.bfloat16

B, T, D = 1, 4096, 2048
H, HKV, DH = 16, 4, 128
GROUP = H // HKV
ROPE_BASE = 10000.0
N_CORES = 8
HL = H // N_CORES  # q-heads per core
KC = D // 128      # contraction tiles for projections
NQ = T // 512      # 512-wide q tiles
NJ = T // 128      # 128-wide kv tiles
NT = T // 128      # 128-row output tiles
NM = D // 512      # 512-wide output column tiles
SCALE = 1.0 / math.sqrt(DH)

Exp = mybir.ActivationFunctionType.Exp
Copy = mybir.ActivationFunctionType.Copy


def _build(nc):
    xp = nc.dram_tensor("xp", [NQ, 128, KC, 512], BF16, kind="ExternalInput").ap()
    wqkv = nc.dram_tensor("wqkv", [128, KC, 4, 128], BF16, kind="ExternalInput").ap()
    wo = nc.dram_tensor("wo", [128, HL, D], BF16, kind="ExternalInput").ap()
    cos2 = nc.dram_tensor("cos2", [128, T], BF16, kind="ExternalInput").ap()
    sinsig = nc.dram_tensor("sinsig", [128, T], BF16, kind="ExternalInput").ap()
    perm = nc.dram_tensor("perm", [128, 128], BF16, kind="ExternalInput").ap()
    y = nc.dram_tensor("y", [T, D], BF16, kind="ExternalOutput").ap()

    with tile.TileContext(nc) as tc, ExitStack() as ctx:
        const = ctx.enter_context(tc.tile_pool(name="const", bufs=1))
        xpool = ctx.enter_context(tc.tile_pool(name="xp", bufs=2))
        psum = ctx.enter_context(tc.tile_pool(name="ps", bufs=4, space="PSUM"))
        opsum = ctx.enter_context(tc.tile_pool(name="ops", bufs=2, space="PSUM"))
        auxp = ctx.enter_context(tc.tile_pool(name="aux", bufs=2, space="PSUM"))
        ppool = ctx.enter_context(tc.tile_pool(name="pt", bufs=6))
        swpool = ctx.enter_context(tc.tile_pool(name="sw", bufs=6))
        spool = ctx.enter_context(tc.tile_pool(name="sm", bufs=4))
        bcpool = ctx.enter_context(tc.tile_pool(name="bc", bufs=2))
        yrow = ctx.enter_context(tc.tile_pool(name="yr", bufs=2))

        wqkv_sb = const.tile([128, KC, 4, 128], BF16, tag="wqkv")
        wo_sb = const.tile([128, HL, D], BF16, tag="wo")
        cos_sb = const.tile([128, T], BF16, tag="cos")
        sin_sb = const.tile([128, T], BF16, tag="sin")
        perm_sb = const.tile([128, 128], BF16, tag="perm")
        ident = const.tile([128, 128], BF16, tag="ident")
        qkvT = const.tile([128, 4, T], BF16, tag="qkvT")   # Q0,Q1,K,V as [d,t]; RoPE in place
        vnat = const.tile([128, NJ, 128], BF16, tag="vnat")  # V natural [j, d]
        ones_sb = const.tile([128, 128], BF16, tag="ones")
        oT = const.tile([128, HL, T], BF16, tag="oT")

        make_identity(nc, ident[:])
        nc.vector.memset(ones_sb[:], 1.0)

        def emit_oproj(oq):
            # partial o_proj y = O^T.T @ Wo for q-tile oq's 4 row blocks;
            # PSUM->SBUF casts alternate DVE/ACT to avoid a single-engine drain.
            for tsub in range(4):
                ti = oq * 4 + tsub
                yr = yrow.tile([128, D], BF16, tag="yr", name=f"yr{ti}")
                for mi in range(NM):
                    yp = psum.tile([128, 512], F32, tag="ps", name=f"yp{ti}_{mi}")
                    for h2 in range(HL):
                        nc.tensor.matmul(
                            yp[:],
                            lhsT=oT[:, h2, bass.ts(ti, 128)],
                            rhs=wo_sb[:, h2, bass.ts(mi, 512)],
                            start=(h2 == 0),
                            stop=(h2 == HL - 1),
                        )
                    if mi % 2 == 0:
                        nc.vector.tensor_copy(yr[:, bass.ts(mi, 512)], yp[:])
                    else:
                        nc.scalar.copy(yr[:, bass.ts(mi, 512)], yp[:])
                nc.gpsimd.dma_start(y[bass.ts(ti, 128), :], yr[:])

        kT = qkvT[:, 2, :]
        for n in range(NQ):
            ns = bass.ts(n, 512)
            # x tile for this 512-token window (split so transfers start early)
            xt = xpool.tile([128, KC, 512], BF16, tag="xt")
            for kq in range(4):
                nc.gpsimd.dma_start(
                    xt[:, bass.ts(kq, 4), :], xp[n, :, bass.ts(kq, 4), :]
                )
                if n == 0:  # interleave weight chunks with the first x tile
                    nc.gpsimd.dma_start(
                        wqkv_sb[:, bass.ts(kq, 4)], wqkv[:, bass.ts(kq, 4)]
                    )
            if n == 0:
                nc.sync.dma_start(perm_sb[:], perm[:])
                nc.sync.dma_start(cos_sb[:], cos2[:])
                nc.sync.dma_start(sin_sb[:], sinsig[:])
                nc.sync.dma_start(wo_sb[:], wo[:])

            # fused q/k/v projection for this window, outputs transposed [d, t]
            for m in range(4):
                ps = psum.tile([128, 512], F32, tag="ps", name=f"prj{n}_{m}")
                for k in range(KC):
                    nc.tensor.matmul(
                        ps[:],
                        lhsT=wqkv_sb[:, k, m, :],
                        rhs=xt[:, k, :],
                        start=(k == 0),
                        stop=(k == KC - 1),
                    )
                nc.scalar.copy(qkvT[:, m, ns], ps[:])

            # RoPE for this window, q heads first: the attention q-tile
            # needs them immediately, while k of THIS window is only read at
            # the tail of the kv loop (jt >= 4n).
            # rotate_half partition swap runs as a permutation matmul on the
            # PE (elementwise engines cannot shift partitions), then
            # src = src*cos + swap(src)*[-sin; sin] in place on DVE.
            for i in (0, 1, 2):
                src = qkvT[:, i, ns]
                sw_ps = psum.tile([128, 512], F32, tag="ps", name=f"swp{n}_{i}")
                nc.tensor.matmul(
                    sw_ps[:], lhsT=perm_sb[:], rhs=src, start=True, stop=True
                )
                swp = swpool.tile([128, 512], BF16, tag="sw", name=f"sw{n}_{i}")
                nc.scalar.copy(swp[:], sw_ps[:])
                nc.vector.tensor_mul(src, src, cos_sb[:, ns])
                nc.vector.tensor_mul(swp[:], swp[:], sin_sb[:, ns])
                nc.vector.tensor_add(src, src, swp[:])

            # V^T -> V natural [j, d] for this window's 4 kv tiles (PE transpose)
            for jt in range(4 * n, 4 * n + 4):
                tp = auxp.tile([128, 128], BF16, tag="aux", name=f"vtp{jt}")
                nc.tensor.transpose(tp[:], qkvT[:, 3, bass.ts(jt, 128)], ident[:])
                nc.vector.tensor_copy(vnat[:, jt, :], tp[:])

            # causal attention for q-tile qi=n, both heads interleaved per kv
            # tile (they share K/V): S^T = K^T.T @ Q^T -> exp on ACT
            # (PSUM->SBUF bf16, fused 1/sqrt(dh)) -> causal mask on diagonal
            # blocks (gpsimd affine_select in place) -> PE accumulations
            # O^T[d,q] += V.T @ P^T and l += ones.T @ P^T (every row of lb
            # is the same column sum) -> O^T normalized by 1/l on DVE.
            qi = n
            njt = 4 * (qi + 1)
            ot = [
                opsum.tile([128, 512], F32, tag="oacc", name=f"oacc{qi}_{h}")
                for h in range(HL)
            ]
            lb = [
                auxp.tile([128, 512], F32, tag="aux", name=f"lacc{qi}_{h}")
                for h in range(HL)
            ]
            for jt in range(njt):
                kd = jt - 4 * qi
                pts = []
                for h in range(HL):
                    sps = psum.tile(
                        [128, 512], F32, tag="ps", name=f"sps{qi}_{jt}_{h}"
                    )
                    nc.tensor.matmul(
                        sps[:],
                        lhsT=kT[:, bass.ts(jt, 128)],
                        rhs=qkvT[:, h, bass.ts(qi, 512)],
                        start=True,
                        stop=True,
                    )
                    pt = ppool.tile([128, 512], BF16, tag="pt", name=f"pt{jt}_{h}")
                    nc.scalar.activation(pt[:], sps[:], Exp, scale=SCALE)
                    if kd >= 0:  # block straddles the diagonal
                        nc.gpsimd.affine_select(
                            out=pt[:],
                            in_=pt[:],
                            compare_op=mybir.AluOpType.is_ge,
                            fill=0.0,
                            base=-kd * 128,
                            channel_multiplier=-1,
                            pattern=[[1, 512]],
                        )
                    pts.append(pt)
                for h in range(HL):
                    nc.tensor.matmul(
                        ot[h][:],
                        lhsT=vnat[:, jt, :],
                        rhs=pts[h][:],
                        start=(jt == 0),
                        stop=(jt == njt - 1),
                    )
                for h in range(HL):
                    nc.tensor.matmul(
                        lb[h][:],
                        lhsT=ones_sb[:],
                        rhs=pts[h][:],
                        start=(jt == 0),
                        stop=(jt == njt - 1),
                    )
            for h in range(HL):
                bc = bcpool.tile([128, 512], F32, tag="bc", name=f"bc{qi}_{h}")
                nc.vector.reciprocal_approx_fast(bc[:], lb[h][:])
                nc.vector.tensor_mul(oT[:, h, bass.ts(qi, 512)], ot[h][:], bc[:])

            # o_proj for the previous q-tile (delayed so the PE has ready
            # work while this q-tile's normalize drains on DVE)
            if qi > 0:
                emit_oproj(qi - 1)
        emit_oproj(NQ - 1)


_CACHE = {}


def _get_program():
    if "nc" not in _CACHE:
        nc = bacc.Bacc(
            "TRN2", target_bir_lowering=False, debug=False, num_devices=N_CORES
        )
        _build(nc)
        nc.compile()
        _CACHE["nc"] = nc
    return _CACHE["nc"]


def _rope_tables():
    inv_freq = 1.0 / (ROPE_BASE ** (np.arange(64, dtype=np.float64) / 64))
    ang = np.arange(T, dtype=np.float64)[:, None] * inv_freq[None, :]  # [T, 64]
    cos = np.cos(ang).T  # [64, T]
    sin = np.sin(ang).T
    cos2 = np.concatenate([cos, cos], axis=0).astype(NPBF16)
    sinsig = np.concatenate([-sin, sin], axis=0).astype(NPBF16)
    return cos2, sinsig


def kernel(x, Wq, Wk, Wv, Wo):
    x = np.asarray(x, dtype=np.float32)
    Wq = np.asarray(Wq, dtype=np.float32)
    Wk = np.asarray(Wk, dtype=np.float32)
    Wv = np.asarray(Wv, dtype=np.float32)
    Wo = np.asarray(Wo, dtype=np.float32)

    # x[t, c] -> xp[n, p, k, j] = x[n*512+j, k*128+p]; contiguous per partition.
    xp = np.ascontiguousarray(
        x.reshape(T, D).reshape(NQ, 512, KC, 128).transpose(0, 3, 2, 1)
    ).astype(NPBF16)
    cos2, sinsig = _rope_tables()
    d_idx = np.arange(128)
    permm = (d_idx[:, None] == (d_idx[None, :] + 64) % 128).astype(NPBF16)

    in_maps = []
    for c in range(N_CORES):
        h0, h1 = 2 * c, 2 * c + 1
        kv = c // 2
        wqkv_c = np.concatenate(
            [
                Wq[:, h0 * DH:(h0 + 1) * DH],
                Wq[:, h1 * DH:(h1 + 1) * DH],
                Wk[:, kv * DH:(kv + 1) * DH],
                Wv[:, kv * DH:(kv + 1) * DH],
            ],
            axis=1,
        )  # [D, 512]
        wqkv_pre = np.ascontiguousarray(
            wqkv_c.reshape(KC, 128, 4, 128).transpose(1, 0, 2, 3)
        ).astype(NPBF16)
        wo_pre = np.ascontiguousarray(
            np.stack(
                [Wo[h0 * DH:(h0 + 1) * DH, :], Wo[h1 * DH:(h1 + 1) * DH, :]], axis=0
            ).transpose(1, 0, 2)
        ).astype(NPBF16)
        in_maps.append(
            {
                "xp": xp,
                "wqkv": wqkv_pre,
                "wo": wo_pre,
                "cos2": cos2,
                "sinsig": sinsig,
                "perm": permm,
            }
        )

    nc = _get_program()
    res = run_bass_kernel_spmd(nc, in_maps, list(range(N_CORES)))
    out = np.zeros((T, D), dtype=np.float32)
    for c in range(N_CORES):
        out += res.results[c]["y"].astype(np.float32)
    return out.reshape(B, T, D)



# revision 12
# speedup vs baseline: 1.0163x; 1.0163x over previous
"""Trainium2 Bass kernel for causal GQA attention (B=1, T=4096, D=2048,
H=16, Hkv=4, Dh=128, RoPE) sharded over 8 NeuronCores.

Sharding: tensor-parallel over heads — each core owns 2 q-heads and the
kv head they share (core c: q-heads {2c, 2c+1}, kv head c//2). Each core
computes its q/k/v projections, RoPE, causal attention and its partial
o_proj contribution y_c = O_c @ Wo_c; the host sums the 8 partials.

On-device dataflow (bf16 into the PE, f32 accumulation):
  xT tiles -> Q^T/K^T/V^T [d,t] (k-outer proj, DVE/ACT drains) ->
  RoPE rotate-half via SBUF-SBUF DMA partition swap + DVE muls ->
  S^T = K^T.T@Q^T per (jt 128, q 512) with causal column trim on
  diagonal blocks -> exp on ACT (PSUM->SBUF bf16, fused 1/sqrt(dh))
  -> diagonal mask on Pool (affine_select, 128-col region only) ->
  O^T[d,q] += V.T @ P^T (PE) and l[q] += ones1.T @ P^T as M=1
  column-tiled matmuls (both heads concurrently in PE col groups
  0 and 32) -> l: copy+recip (DVE), partition broadcast (Pool),
  O^T normalize (DVE) -> y = O^T.T @ Wo, drains DVE/ACT alternated.
"""

import sys

sys.path.insert(0, "/opt/trn_rl_repo")

import math
from contextlib import ExitStack

import ml_dtypes
import numpy as np

import concourse.bass as bass
import concourse.tile as tile
from concourse import bacc, mybir
from concourse.bass_utils import run_bass_kernel_spmd
from concourse.masks import make_identity

BF16 = mybir.dt.bfloat16
F32 = mybir.dt.float32
NPBF16 = ml_dtypes.bfloat16

B, T, D = 1, 4096, 2048
H, HKV, DH = 16, 4, 128
GROUP = H // HKV
ROPE_BASE = 10000.0
N_CORES = 8
HL = H // N_CORES  # q-heads per core
KC = D // 128      # contraction tiles for projections
NQ = T // 512      # 512-wide q tiles
NJ = T // 128      # 128-wide kv tiles
NT = T // 128      # 128-row output tiles
NM = D // 512      # 512-wide output column tiles
SCALE = 1.0 / math.sqrt(DH)

Exp = mybir.ActivationFunctionType.Exp
Copy = mybir.ActivationFunctionType.Copy

import os

# M=1 column-tiled softmax-denominator matmuls + Pool partition_broadcast
# normalize; when off, falls back to full ones[128,128] matmuls + DVE
# reciprocal like the original kernel.
COLTILE_L = os.environ.get("K_COLTILE_L", "1") == "1"
DMA_ROPE = os.environ.get("K_DMA_ROPE", "1") == "1"


def _build(nc):
    xp = nc.dram_tensor("xp", [NQ, 128, KC, 512], BF16, kind="ExternalInput").ap()
    wqkv = nc.dram_tensor("wqkv", [128, KC, 4, 128], BF16, kind="ExternalInput").ap()
    wo = nc.dram_tensor("wo", [128, HL, D], BF16, kind="ExternalInput").ap()
    cos2 = nc.dram_tensor("cos2", [128, T], BF16, kind="ExternalInput").ap()
    sinsig = nc.dram_tensor("sinsig", [128, T], BF16, kind="ExternalInput").ap()
    y = nc.dram_tensor("y", [T, D], BF16, kind="ExternalOutput").ap()

    with tile.TileContext(nc) as tc, ExitStack() as ctx:
        const = ctx.enter_context(tc.tile_pool(name="const", bufs=1))
        xpool = ctx.enter_context(tc.tile_pool(name="xp", bufs=2))
        psum = ctx.enter_context(tc.tile_pool(name="ps", bufs=4, space="PSUM"))
        opsum = ctx.enter_context(tc.tile_pool(name="ops", bufs=2, space="PSUM"))
        laux = ctx.enter_context(tc.tile_pool(name="laux", bufs=2, space="PSUM"))
        ppool = ctx.enter_context(tc.tile_pool(name="pt", bufs=6))
        swpool = ctx.enter_context(tc.tile_pool(name="sw", bufs=6))
        lspool = ctx.enter_context(tc.tile_pool(name="ls", bufs=2))
        lbpool = ctx.enter_context(tc.tile_pool(name="lb", bufs=2))
        yrow = ctx.enter_context(tc.tile_pool(name="yr", bufs=2))

        wqkv_sb = const.tile([128, KC, 4, 128], BF16, tag="wqkv")
        wo_sb = const.tile([128, HL, D], BF16, tag="wo")
        cos_sb = const.tile([128, T], BF16, tag="cos")
        sin_sb = const.tile([128, T], BF16, tag="sin")
        ident = const.tile([128, 128], BF16, tag="ident")
        qkvT = const.tile([128, 4, T], BF16, tag="qkvT")   # Q0,Q1,K,V as [d,t]
        vnat = const.tile([128, NJ, 128], BF16, tag="vnat")  # V natural [j, d]
        ones_sb = const.tile([128, 128], BF16, tag="ones")
        oT = const.tile([128, HL, T], BF16, tag="oT")

        make_identity(nc, ident[:])
        nc.vector.memset(ones_sb[:], 1.0)
        if not DMA_ROPE:
            # rotate-half permutation: perm[:, j] = ident[:, (j+64)%128]
            perm_sb = const.tile([128, 128], BF16, tag="perm")
            nc.vector.tensor_copy(perm_sb[:, 0:64], ident[:, 64:128])
            nc.vector.tensor_copy(perm_sb[:, 64:128], ident[:, 0:64])

        def emit_oproj(oq):
            # partial o_proj y = O^T.T @ Wo for q-tile oq's 4 row blocks.
            # h2-outer so each oT row block is loaded as weights only twice;
            # PSUM->SBUF casts alternate DVE/ACT to avoid a single-engine
            # drain (ACT's exp backlog builds later in the window).
            for tsub in range(4):
                ti = oq * 4 + tsub
                yr = yrow.tile([128, D], BF16, tag="yr", name=f"yr{ti}")
                yps = [
                    psum.tile([128, 512], F32, tag="ps", name=f"yp{ti}_{mi}")
                    for mi in range(NM)
                ]
                for h2 in range(HL):
                    for mi in range(NM):
                        nc.tensor.matmul(
                            yps[mi][:],
                            lhsT=oT[:, h2, bass.ts(ti, 128)],
                            rhs=wo_sb[:, h2, bass.ts(mi, 512)],
                            start=(h2 == 0),
                            stop=(h2 == HL - 1),
                        )
                for mi in range(NM):
                    if mi % 2 == 0:
                        nc.vector.tensor_copy(yr[:, bass.ts(mi, 512)], yps[mi][:])
                    else:
                        nc.scalar.copy(yr[:, bass.ts(mi, 512)], yps[mi][:])
                nc.sync.dma_start(y[bass.ts(ti, 128), :], yr[:])

        kT = qkvT[:, 2, :]
        for n in range(NQ):
            ns = bass.ts(n, 512)
            # x tile for this 512-token window. Window 0 uses fine-grained
            # chunks interleaved with the weight DMA so the first projection
            # matmuls can start after ~0.5 MB instead of ~2 MB.
            xt = xpool.tile([128, KC, 512], BF16, tag="xt")
            if n == 0:
                nc.sync.dma_start(cos_sb[:], cos2[:])
                nc.sync.dma_start(sin_sb[:], sinsig[:])
                for kq in range(8):
                    nc.gpsimd.dma_start(
                        xt[:, bass.ts(kq, 2), :], xp[n, :, bass.ts(kq, 2), :]
                    )
                    nc.gpsimd.dma_start(
                        wqkv_sb[:, bass.ts(kq, 2)], wqkv[:, bass.ts(kq, 2)]
                    )
                nc.sync.dma_start(wo_sb[:], wo[:])
            else:
                for kq in range(4):
                    nc.gpsimd.dma_start(
                        xt[:, bass.ts(kq, 4), :], xp[n, :, bass.ts(kq, 4), :]
                    )

            # fused q/k/v projection, outputs transposed [d, t]; k-outer so
            # each arriving x chunk immediately feeds 4 matmuls.
            prj = [
                psum.tile([128, 512], F32, tag="ps", name=f"prj{n}_{m}")
                for m in range(4)
            ]
            for k in range(KC):
                for m in range(4):
                    nc.tensor.matmul(
                        prj[m][:],
                        lhsT=wqkv_sb[:, k, m, :],
                        rhs=xt[:, k, :],
                        start=(k == 0),
                        stop=(k == KC - 1),
                    )
            for m in range(4):
                if m % 2 == 0:
                    nc.vector.tensor_copy(qkvT[:, m, ns], prj[m][:])
                else:
                    nc.scalar.copy(qkvT[:, m, ns], prj[m][:])

            # RoPE rotate-half: partition swap via two SBUF->SBUF DMAs (no
            # PE permutation matmul, no ACT drain), then
            # src = src*cos + swap(src)*[-sin; sin] in place on DVE.
            for i in (0, 1, 2):
                src = qkvT[:, i, ns]
                swp = swpool.tile([128, 512], BF16, tag="sw", name=f"sw{n}_{i}")
                if DMA_ROPE:
                    nc.sync.dma_start(swp[0:64, :], qkvT[64:128, i, ns])
                    nc.sync.dma_start(swp[64:128, :], qkvT[0:64, i, ns])
                else:
                    sw_ps = psum.tile([128, 512], F32, tag="ps", name=f"swp{n}_{i}")
                    nc.tensor.matmul(
                        sw_ps[:], lhsT=perm_sb[:], rhs=src, start=True, stop=True
                    )
                    nc.scalar.copy(swp[:], sw_ps[:])
                nc.vector.tensor_mul(src, src, cos_sb[:, ns])
                nc.vector.tensor_mul(swp[:], swp[:], sin_sb[:, ns])
                nc.vector.tensor_add(src, src, swp[:])

            # o_proj for the previous q-tile: placed here so the PE has
            # dense work while the rope DMAs + DVE chain complete.
            if n > 0:
                emit_oproj(n - 1)

            # V^T -> V natural [j, d] for this window's 4 kv tiles
            for jt in range(4 * n, 4 * n + 4):
                tp = psum.tile([128, 128], BF16, tag="ps", name=f"vtp{jt}")
                nc.tensor.transpose(tp[:], qkvT[:, 3, bass.ts(jt, 128)], ident[:])
                nc.vector.tensor_copy(vnat[:, jt, :], tp[:])

            # causal attention for q-tile qi=n, both heads interleaved per
            # kv tile (they share K/V). Diagonal blocks are column-trimmed:
            # block jt touches only q columns >= 128*kd (kd = jt-4qi), so
            # S / exp / O / l all run on the [s0:512] subrange and the mask
            # only needs the first 128 columns of that subrange.
            qi = n
            njt = 4 * (qi + 1)
            ot = [
                opsum.tile([128, 512], F32, tag="oacc", name=f"oacc{qi}_{h}")
                for h in range(HL)
            ]
            # softmax denominators: head h accumulates an M=1 row at
            # partition 32*h via column-tiled matmuls (PE col groups 0 and
            # 32 run them concurrently); separate banks per head so each
            # accumulation group owns its bank.
            lb = [
                laux.tile([128, 512], F32, tag="laux", name=f"lacc{qi}_{h}")
                for h in range(HL)
            ]
            # Software-pipelined by one kv block: O/l accumulation for block
            # jt is emitted after the S matmuls of block jt+1, so the PE has
            # S streams to chew on while ACT exp + Pool mask of block jt
            # complete.
            pend = None  # (jt, pts) awaiting O/l emission

            def emit_s(jt):
                kd = jt - 4 * qi
                s0 = 128 * kd if kd > 0 else 0
                w = 512 - s0
                pts = []
                for h in range(HL):
                    sps = psum.tile(
                        [128, 512], F32, tag="ps", name=f"sps{qi}_{jt}_{h}"
                    )
                    nc.tensor.matmul(
                        sps[:, :w],
                        lhsT=kT[:, bass.ts(jt, 128)],
                        rhs=qkvT[:, h, qi * 512 + s0 : (qi + 1) * 512],
                        start=True,
                        stop=True,
                    )
                    pt = ppool.tile([128, 512], BF16, tag="pt", name=f"pt{jt}_{h}")
                    nc.scalar.activation(pt[:, :w], sps[:, :w], Exp, scale=SCALE)
                    if kd >= 0:  # mask the 128-col block straddling the diagonal
                        nc.gpsimd.affine_select(
                            out=pt[:, :128],
                            in_=pt[:, :128],
                            compare_op=mybir.AluOpType.is_ge,
                            fill=0.0,
                            base=0,
                            channel_multiplier=-1,
                            pattern=[[1, 128]],
                        )
                    pts.append((pt, s0, w))
                return pts

            def emit_ol(jt, pts):
                for h in range(HL):
                    pt, ps0, pw = pts[h]
                    nc.tensor.matmul(
                        ot[h][:, ps0:],
                        lhsT=vnat[:, jt, :],
                        rhs=pt[:, :pw],
                        start=(jt == 0),
                        stop=(jt == njt - 1),
                    )
                for h in range(HL):
                    pt, ps0, pw = pts[h]
                    if COLTILE_L:
                        nc.tensor.matmul(
                            lb[h][32 * h : 32 * h + 1, ps0:],
                            lhsT=ones_sb[:, :1],
                            rhs=pt[:, :pw],
                            start=(jt == 0),
                            stop=(jt == njt - 1),
                        )
                    else:
                        nc.tensor.matmul(
                            lb[h][:, ps0:],
                            lhsT=ones_sb[:],
                            rhs=pt[:, :pw],
                            start=(jt == 0),
                            stop=(jt == njt - 1),
                        )
            if COLTILE_L:
                # normalize: 1/l broadcast across partitions (Pool), O^T
                # scaled on DVE
                lsb = lspool.tile([128, 512], F32, tag="ls", name=f"ls{qi}")
                for h in range(HL):
                    p = 32 * h
                    nc.vector.tensor_copy(lsb[p : p + 1, :], lb[h][p : p + 1, :])
                    nc.vector.reciprocal_approx_fast(
                        lsb[p : p + 1, :], lsb[p : p + 1, :]
                    )
                    lbc = lbpool.tile([128, 512], F32, tag="lb", name=f"lbc{qi}_{h}")
                    nc.gpsimd.partition_broadcast(lbc[:], lsb[p : p + 1, :])
                    nc.vector.tensor_mul(oT[:, h, ns], ot[h][:], lbc[:])
            else:
                for h in range(HL):
                    lbc = lbpool.tile([128, 512], F32, tag="lb", name=f"lbc{qi}_{h}")
                    nc.vector.reciprocal_approx_fast(lbc[:], lb[h][:])
                    nc.vector.tensor_mul(oT[:, h, ns], ot[h][:], lbc[:])
        emit_oproj(NQ - 1)


_CACHE = {}


def _get_program():
    if "nc" not in _CACHE:
        nc = bacc.Bacc(
            "TRN2", target_bir_lowering=False, debug=False, num_devices=N_CORES
        )
        _build(nc)
        nc.compile()
        _CACHE["nc"] = nc
    return _CACHE["nc"]


def _rope_tables():
    inv_freq = 1.0 / (ROPE_BASE ** (np.arange(64, dtype=np.float64) / 64))
    ang = np.arange(T, dtype=np.float64)[:, None] * inv_freq[None, :]  # [T, 64]
    cos = np.cos(ang).T  # [64, T]
    sin = np.sin(ang).T
    cos2 = np.concatenate([cos, cos], axis=0).astype(NPBF16)
    sinsig = np.concatenate([-sin, sin], axis=0).astype(NPBF16)
    return cos2, sinsig


def kernel(x, Wq, Wk, Wv, Wo):
    x = np.asarray(x, dtype=np.float32)
    Wq = np.asarray(Wq, dtype=np.float32)
    Wk = np.asarray(Wk, dtype=np.float32)
    Wv = np.asarray(Wv, dtype=np.float32)
    Wo = np.asarray(Wo, dtype=np.float32)

    # x[t, c] -> xp[n, p, k, j] = x[n*512+j, k*128+p]; contiguous per partition.
    xp = np.ascontiguousarray(
        x.reshape(T, D).reshape(NQ, 512, KC, 128).transpose(0, 3, 2, 1)
    ).astype(NPBF16)
    cos2, sinsig = _rope_tables()

    in_maps = []
    for c in range(N_CORES):
        h0, h1 = 2 * c, 2 * c + 1
        kv = c // 2
        wqkv_c = np.concatenate(
            [
                Wq[:, h0 * DH:(h0 + 1) * DH],
                Wq[:, h1 * DH:(h1 + 1) * DH],
                Wk[:, kv * DH:(kv + 1) * DH],
                Wv[:, kv * DH:(kv + 1) * DH],
            ],
            axis=1,
        )  # [D, 512]
        wqkv_pre = np.ascontiguousarray(
            wqkv_c.reshape(KC, 128, 4, 128).transpose(1, 0, 2, 3)
        ).astype(NPBF16)
        wo_pre = np.ascontiguousarray(
            np.stack(
                [Wo[h0 * DH:(h0 + 1) * DH, :], Wo[h1 * DH:(h1 + 1) * DH, :]], axis=0
            ).transpose(1, 0, 2)
        ).astype(NPBF16)
        in_maps.append(
            {
                "xp": xp,
                "wqkv": wqkv_pre,
                "wo": wo_pre,
                "cos2": cos2,
                "sinsig": sinsig,
            }
        )

    nc = _get_program()
    res = run_bass_kernel_spmd(nc, in_maps, list(range(N_CORES)))
    out = np.zeros((T, D), dtype=np.float32)
    for c in range(N_CORES):
        out += res.results[c]["y"].astype(np.float32)
    return out.reshape(B, T, D)


# revision 14
# speedup vs baseline: 1.0166x; 1.0003x over previous
"""Trainium2 Bass kernel for causal GQA attention (B=1, T=4096, D=2048,
H=16, Hkv=4, Dh=128, RoPE) sharded over 8 NeuronCores.

Sharding: tensor-parallel over heads — each core owns 2 q-heads and the
kv head they share (core c: q-heads {2c, 2c+1}, kv head c//2). Each core
computes its q/k/v projections, RoPE, causal attention and its partial
o_proj contribution y_c = O_c @ Wo_c; the host sums the 8 partials.

On-device dataflow (bf16 into the PE, f32 accumulation):
  xT tiles -> Q^T/K^T/V^T [d,t] (k-outer proj, DVE/ACT drains) ->
  RoPE rotate-half via SBUF-SBUF DMA partition swap + DVE muls ->
  S^T = K^T.T@Q^T per (jt 128, q 512) with causal column trim on
  diagonal blocks -> exp on ACT (PSUM->SBUF bf16, fused 1/sqrt(dh))
  -> diagonal mask on Pool (affine_select, 128-col region only) ->
  O^T[d,q] += V.T @ P^T (PE) and l[q] += ones1.T @ P^T as M=1
  column-tiled matmuls (both heads concurrently in PE col groups
  0 and 32) -> l: copy+recip (DVE), partition broadcast (Pool),
  O^T normalize (DVE) -> y = O^T.T @ Wo, drains DVE/ACT alternated.
"""

import sys

sys.path.insert(0, "/opt/trn_rl_repo")

import math
from contextlib import ExitStack

import ml_dtypes
import numpy as np

import concourse.bass as bass
import concourse.tile as tile
from concourse import bacc, mybir
from concourse.bass_utils import run_bass_kernel_spmd
from concourse.masks import make_identity

BF16 = mybir.dt.bfloat16
F32 = mybir.dt.float32
NPBF16 = ml_dtypes.bfloat16

B, T, D = 1, 4096, 2048
H, HKV, DH = 16, 4, 128
GROUP = H // HKV
ROPE_BASE = 10000.0
N_CORES = 8
HL = H // N_CORES  # q-heads per core
KC = D // 128      # contraction tiles for projections
NQ = T // 512      # 512-wide q tiles
NJ = T // 128      # 128-wide kv tiles
NT = T // 128      # 128-row output tiles
NM = D // 512      # 512-wide output column tiles
SCALE = 1.0 / math.sqrt(DH)

Exp = mybir.ActivationFunctionType.Exp
Copy = mybir.ActivationFunctionType.Copy

import os

# M=1 column-tiled softmax-denominator matmuls + Pool partition_broadcast
# normalize; when off, falls back to full ones[128,128] matmuls + DVE
# reciprocal like the original kernel.
COLTILE_L = os.environ.get("K_COLTILE_L", "1") == "1"
DMA_ROPE = os.environ.get("K_DMA_ROPE", "1") == "1"


def _build(nc):
    xp = nc.dram_tensor("xp", [NQ, 128, KC, 512], BF16, kind="ExternalInput").ap()
    wqkv = nc.dram_tensor("wqkv", [128, KC, 4, 128], BF16, kind="ExternalInput").ap()
    wo = nc.dram_tensor("wo", [128, HL, D], BF16, kind="ExternalInput").ap()
    cos2 = nc.dram_tensor("cos2", [128, T], BF16, kind="ExternalInput").ap()
    sinsig = nc.dram_tensor("sinsig", [128, T], BF16, kind="ExternalInput").ap()
    y = nc.dram_tensor("y", [T, D], BF16, kind="ExternalOutput").ap()

    with tile.TileContext(nc) as tc, ExitStack() as ctx:
        const = ctx.enter_context(tc.tile_pool(name="const", bufs=1))
        xpool = ctx.enter_context(tc.tile_pool(name="xp", bufs=2))
        psum = ctx.enter_context(tc.tile_pool(name="ps", bufs=4, space="PSUM"))
        opsum = ctx.enter_context(tc.tile_pool(name="ops", bufs=2, space="PSUM"))
        laux = ctx.enter_context(tc.tile_pool(name="laux", bufs=2, space="PSUM"))
        ppool = ctx.enter_context(tc.tile_pool(name="pt", bufs=6))
        swpool = ctx.enter_context(tc.tile_pool(name="sw", bufs=6))
        lspool = ctx.enter_context(tc.tile_pool(name="ls", bufs=2))
        lbpool = ctx.enter_context(tc.tile_pool(name="lb", bufs=2))
        yrow = ctx.enter_context(tc.tile_pool(name="yr", bufs=2))

        wqkv_sb = const.tile([128, KC, 4, 128], BF16, tag="wqkv")
        wo_sb = const.tile([128, HL, D], BF16, tag="wo")
        cos_sb = const.tile([128, T], BF16, tag="cos")
        sin_sb = const.tile([128, T], BF16, tag="sin")
        ident = const.tile([128, 128], BF16, tag="ident")
        qkvT = const.tile([128, 4, T], BF16, tag="qkvT")   # Q0,Q1,K,V as [d,t]
        vnat = const.tile([128, NJ, 128], BF16, tag="vnat")  # V natural [j, d]
        ones_sb = const.tile([128, 128], BF16, tag="ones")
        oT = const.tile([128, HL, T], BF16, tag="oT")

        make_identity(nc, ident[:])
        nc.vector.memset(ones_sb[:], 1.0)
        if not DMA_ROPE:
            # rotate-half permutation: perm[:, j] = ident[:, (j+64)%128]
            perm_sb = const.tile([128, 128], BF16, tag="perm")
            nc.vector.tensor_copy(perm_sb[:, 0:64], ident[:, 64:128])
            nc.vector.tensor_copy(perm_sb[:, 64:128], ident[:, 0:64])

        def emit_oproj(oq):
            # partial o_proj y = O^T.T @ Wo for q-tile oq's 4 row blocks.
            # h2-outer so each oT row block is loaded as weights only twice;
            # PSUM->SBUF casts alternate DVE/ACT to avoid a single-engine
            # drain (ACT's exp backlog builds later in the window).
            for tsub in range(4):
                ti = oq * 4 + tsub
                yr = yrow.tile([128, D], BF16, tag="yr", name=f"yr{ti}")
                yps = [
                    psum.tile([128, 512], F32, tag="ps", name=f"yp{ti}_{mi}")
                    for mi in range(NM)
                ]
                for h2 in range(HL):
                    for mi in range(NM):
                        nc.tensor.matmul(
                            yps[mi][:],
                            lhsT=oT[:, h2, bass.ts(ti, 128)],
                            rhs=wo_sb[:, h2, bass.ts(mi, 512)],
                            start=(h2 == 0),
                            stop=(h2 == HL - 1),
                        )
                for mi in range(NM):
                    if mi % 2 == 0:
                        nc.vector.tensor_copy(yr[:, bass.ts(mi, 512)], yps[mi][:])
                    else:
                        nc.scalar.copy(yr[:, bass.ts(mi, 512)], yps[mi][:])
                nc.sync.dma_start(y[bass.ts(ti, 128), :], yr[:])

        kT = qkvT[:, 2, :]
        xts = {}

        def issue_xt(n):
            # prefetch window n's x tile; called one window ahead so the
            # ~20us transfer overlaps the previous window's attention.
            xt = xpool.tile([128, KC, 512], BF16, tag="xt", name=f"xt{n}")
            nch = 8 if n == 0 else 4
            step = KC // nch
            for kq in range(nch):
                nc.gpsimd.dma_start(
                    xt[:, bass.ts(kq, step), :], xp[n, :, bass.ts(kq, step), :]
                )
                if n == 0:  # interleave weight chunks with the first x tile
                    nc.gpsimd.dma_start(
                        wqkv_sb[:, bass.ts(kq, 2)], wqkv[:, bass.ts(kq, 2)]
                    )
            xts[n] = xt

        for n in range(NQ):
            ns = bass.ts(n, 512)
            if n == 0:
                nc.sync.dma_start(cos_sb[:], cos2[:])
                nc.sync.dma_start(sin_sb[:], sinsig[:])
                issue_xt(0)
                nc.sync.dma_start(wo_sb[:], wo[:])
            xt = xts.pop(n)

            # fused q/k/v projection into raw (pre-RoPE) tiles, [d, t];
            # k-outer so each arriving x chunk immediately feeds 4 matmuls.
            prj = [
                psum.tile([128, 512], F32, tag="ps", name=f"prj{n}_{m}")
                for m in range(4)
            ]
            for k in range(KC):
                for m in range(4):
                    nc.tensor.matmul(
                        prj[m][:],
                        lhsT=wqkv_sb[:, k, m, :],
                        rhs=xt[:, k, :],
                        start=(k == 0),
                        stop=(k == KC - 1),
                    )
            if n + 1 < NQ:
                issue_xt(n + 1)
            qraw = swpool.tile([128, 3, 512], BF16, tag="qraw", name=f"qraw{n}")
            for m in range(4):
                dst = qkvT[:, m, ns] if m == 3 else qraw[:, m, :]
                if m % 2 == 0:
                    nc.vector.tensor_copy(dst, prj[m][:])
                else:
                    nc.scalar.copy(dst, prj[m][:])

            # RoPE rotate-half: partition swap via two SBUF->SBUF DMAs (no
            # PE permutation matmul, no ACT drain). Writing the cos product
            # into qkvT (not in place) keeps the DVE chain off the DMA's
            # write-after-read path.
            for i in (0, 1, 2):
                src = qraw[:, i, :]
                dst = qkvT[:, i, ns]
                swp = swpool.tile([128, 512], BF16, tag="sw", name=f"sw{n}_{i}")
                if DMA_ROPE:
                    nc.sync.dma_start(swp[0:64, :], qraw[64:128, i, :])
                    nc.sync.dma_start(swp[64:128, :], qraw[0:64, i, :])
                else:
                    sw_ps = psum.tile([128, 512], F32, tag="ps", name=f"swp{n}_{i}")
                    nc.tensor.matmul(
                        sw_ps[:], lhsT=perm_sb[:], rhs=src, start=True, stop=True
                    )
                    nc.scalar.copy(swp[:], sw_ps[:])
                nc.vector.tensor_mul(dst, src, cos_sb[:, ns])
                nc.vector.tensor_mul(swp[:], swp[:], sin_sb[:, ns])
                nc.vector.tensor_add(dst, dst, swp[:])

            # o_proj for the previous q-tile: placed here so the PE has
            # dense work while the rope DMAs + DVE chain complete.
            if n > 0:
                emit_oproj(n - 1)

            # V^T -> V natural [j, d] for this window's 4 kv tiles
            for jt in range(4 * n, 4 * n + 4):
                tp = psum.tile([128, 128], BF16, tag="ps", name=f"vtp{jt}")
                nc.tensor.transpose(tp[:], qkvT[:, 3, bass.ts(jt, 128)], ident[:])
                nc.vector.tensor_copy(vnat[:, jt, :], tp[:])

            # causal attention for q-tile qi=n, both heads interleaved per
            # kv tile (they share K/V). Diagonal blocks are column-trimmed:
            # block jt touches only q columns >= 128*kd (kd = jt-4qi), so
            # S / exp / O / l all run on the [s0:512] subrange and the mask
            # only needs the first 128 columns of that subrange.
            qi = n
            njt = 4 * (qi + 1)
            ot = [
                opsum.tile([128, 512], F32, tag="oacc", name=f"oacc{qi}_{h}")
                for h in range(HL)
            ]
            # softmax denominators: head h accumulates an M=1 row at
            # partition 32*h via column-tiled matmuls (PE col groups 0 and
            # 32 run them concurrently); separate banks per head so each
            # accumulation group owns its bank.
            lb = [
                laux.tile([128, 512], F32, tag="laux", name=f"lacc{qi}_{h}")
                for h in range(HL)
            ]
            # Software-pipelined by one kv block: O/l accumulation for block
            # jt is emitted after the S matmuls of block jt+1, so the PE has
            # S streams to chew on while ACT exp + Pool mask of block jt
            # complete.
            pend = None  # (jt, pts) awaiting O/l emission

            def emit_s(jt):
                kd = jt - 4 * qi
                s0 = 128 * kd if kd > 0 else 0
                w = 512 - s0
                pts = []
                for h in range(HL):
                    sps = psum.tile(
                        [128, 512], F32, tag="ps", name=f"sps{qi}_{jt}_{h}"
                    )
                    nc.tensor.matmul(
                        sps[:, :w],
                        lhsT=kT[:, bass.ts(jt, 128)],
                        rhs=qkvT[:, h, qi * 512 + s0 : (qi + 1) * 512],
                        start=True,
                        stop=True,
                    )
                    pt = ppool.tile([128, 512], BF16, tag="pt", name=f"pt{jt}_{h}")
                    nc.scalar.activation(pt[:, :w], sps[:, :w], Exp, scale=SCALE)
                    if kd >= 0:  # mask the 128-col block straddling the diagonal
                        nc.gpsimd.affine_select(
                            out=pt[:, :128],
                            in_=pt[:, :128],
                            compare_op=mybir.AluOpType.is_ge,
                            fill=0.0,
                            base=0,
                            channel_multiplier=-1,
                            pattern=[[1, 128]],
                        )
                    pts.append((pt, s0, w))
                return pts

            def emit_ol(jt, pts):
                for h in range(HL):
                    pt, ps0, pw = pts[h]
                    nc.tensor.matmul(
                        ot[h][:, ps0:],
                        lhsT=vnat[:, jt, :],
                        rhs=pt[:, :pw],
                        start=(jt == 0),
                        stop=(jt == njt - 1),
                    )
                for h in range(HL):
                    pt, ps0, pw = pts[h]
                    if COLTILE_L:
                        nc.tensor.matmul(
                            lb[h][32 * h : 32 * h + 1, ps0:],
                            lhsT=ones_sb[:, :1],
                            rhs=pt[:, :pw],
                            start=(jt == 0),
                            stop=(jt == njt - 1),
                        )
                    else:
                        nc.tensor.matmul(
                            lb[h][:, ps0:],
                            lhsT=ones_sb[:],
                            rhs=pt[:, :pw],
                            start=(jt == 0),
                            stop=(jt == njt - 1),
                        )

            for jt in range(njt):
                pts = emit_s(jt)
                if pend is not None:
                    emit_ol(*pend)
                pend = (jt, pts)
            emit_ol(*pend)

            if COLTILE_L:
                # normalize: 1/l broadcast across partitions (Pool), O^T
                # scaled on DVE
                lsb = lspool.tile([128, 512], F32, tag="ls", name=f"ls{qi}")
                for h in range(HL):
                    p = 32 * h
                    nc.vector.tensor_copy(lsb[p : p + 1, :], lb[h][p : p + 1, :])
                    nc.vector.reciprocal_approx_fast(
                        lsb[p : p + 1, :], lsb[p : p + 1, :]
                    )
                    lbc = lbpool.tile([128, 512], F32, tag="lb", name=f"lbc{qi}_{h}")
                    nc.gpsimd.partition_broadcast(lbc[:], lsb[p : p + 1, :])
                    nc.vector.tensor_mul(oT[:, h, ns], ot[h][:], lbc[:])
            else:
                for h in range(HL):
                    lbc = lbpool.tile([128, 512], F32, tag="lb", name=f"lbc{qi}_{h}")
                    nc.vector.reciprocal_approx_fast(lbc[:], lb[h][:])
                    nc.vector.tensor_mul(oT[:, h, ns], ot[h][:], lbc[:])
        emit_oproj(NQ - 1)


_CACHE = {}


def _get_program():
    if "nc" not in _CACHE:
        nc = bacc.Bacc(
            "TRN2", target_bir_lowering=False, debug=False, num_devices=N_CORES
        )
        _build(nc)
        nc.compile()
        _CACHE["nc"] = nc
    return _CACHE["nc"]


def _rope_tables():
    inv_freq = 1.0 / (ROPE_BASE ** (np.arange(64, dtype=np.float64) / 64))
    ang = np.arange(T, dtype=np.float64)[:, None] * inv_freq[None, :]  # [T, 64]
    cos = np.cos(ang).T  # [64, T]
    sin = np.sin(ang).T
    cos2 = np.concatenate([cos, cos], axis=0).astype(NPBF16)
    sinsig = np.concatenate([-sin, sin], axis=0).astype(NPBF16)
    return cos2, sinsig


def kernel(x, Wq, Wk, Wv, Wo):
    x = np.asarray(x, dtype=np.float32)
    Wq = np.asarray(Wq, dtype=np.float32)
    Wk = np.asarray(Wk, dtype=np.float32)
    Wv = np.asarray(Wv, dtype=np.float32)
    Wo = np.asarray(Wo, dtype=np.float32)

    # x[t, c] -> xp[n, p, k, j] = x[n*512+j, k*128+p]; contiguous per partition.
    xp = np.ascontiguousarray(
        x.reshape(T, D).reshape(NQ, 512, KC, 128).transpose(0, 3, 2, 1)
    ).astype(NPBF16)
    cos2, sinsig = _rope_tables()

    in_maps = []
    for c in range(N_CORES):
        h0, h1 = 2 * c, 2 * c + 1
        kv = c // 2
        wqkv_c = np.concatenate(
            [
                Wq[:, h0 * DH:(h0 + 1) * DH],
                Wq[:, h1 * DH:(h1 + 1) * DH],
                Wk[:, kv * DH:(kv + 1) * DH],
                Wv[:, kv * DH:(kv + 1) * DH],
            ],
            axis=1,
        )  # [D, 512]
        wqkv_pre = np.ascontiguousarray(
            wqkv_c.reshape(KC, 128, 4, 128).transpose(1, 0, 2, 3)
        ).astype(NPBF16)
        wo_pre = np.ascontiguousarray(
            np.stack(
                [Wo[h0 * DH:(h0 + 1) * DH, :], Wo[h1 * DH:(h1 + 1) * DH, :]], axis=0
            ).transpose(1, 0, 2)
        ).astype(NPBF16)
        in_maps.append(
            {
                "xp": xp,
                "wqkv": wqkv_pre,
                "wo": wo_pre,
                "cos2": cos2,
                "sinsig": sinsig,
            }
        )

    nc = _get_program()
    res = run_bass_kernel_spmd(nc, in_maps, list(range(N_CORES)))
    out = np.zeros((T, D), dtype=np.float32)
    for c in range(N_CORES):
        out += res.results[c]["y"].astype(np.float32)
    return out.reshape(B, T, D)


# revision 20
# speedup vs baseline: 1.0263x; 1.0095x over previous
"""Trainium2 Bass kernel for causal GQA attention (B=1, T=4096, D=2048,
H=16, Hkv=4, Dh=128, RoPE) sharded over 8 NeuronCores.

Sharding: tensor-parallel over heads — each core owns 2 q-heads and the
kv head they share (core c: q-heads {2c, 2c+1}, kv head c//2). Each core
computes its q/k/v projections, RoPE, causal attention and its partial
o_proj contribution y_c = O_c @ Wo_c; the host sums the 8 partials.

On-device dataflow (bf16 into the PE, f32 accumulation):
  xT tiles -> Q^T/K^T/V^T [d,t] (k-outer proj, DVE/ACT drains) ->
  RoPE rotate-half via SBUF-SBUF DMA partition swap + DVE muls ->
  S^T = K^T.T@Q^T per (jt 128, q 512) with causal column trim on
  diagonal blocks -> exp on ACT (PSUM->SBUF bf16, fused 1/sqrt(dh))
  -> diagonal mask on Pool (affine_select, 128-col region only) ->
  O^T[d,q] += V.T @ P^T (PE) and l[q] += ones1.T @ P^T as M=1
  column-tiled matmuls (both heads concurrently in PE col groups
  0 and 32) -> l: copy+recip (DVE), partition broadcast (Pool),
  O^T normalize (DVE) -> y = O^T.T @ Wo, drains DVE/ACT alternated.
"""

import sys

sys.path.insert(0, "/opt/trn_rl_repo")

import math
from contextlib import ExitStack

import ml_dtypes
import numpy as np

import concourse.bass as bass
import concourse.tile as tile
from concourse import bacc, mybir
from concourse.bass_utils import run_bass_kernel_spmd
from concourse.masks import make_identity

BF16 = mybir.dt.bfloat16
F32 = mybir.dt.float32
NPBF16 = ml_dtypes.bfloat16

B, T, D = 1, 4096, 2048
H, HKV, DH = 16, 4, 128
GROUP = H // HKV
ROPE_BASE = 10000.0
N_CORES = 8
HL = H // N_CORES  # q-heads per core
KC = D // 128      # contraction tiles for projections
NQ = T // 512      # 512-wide q tiles
NJ = T // 128      # 128-wide kv tiles
NT = T // 128      # 128-row output tiles
NM = D // 512      # 512-wide output column tiles
SCALE = 1.0 / math.sqrt(DH)

Exp = mybir.ActivationFunctionType.Exp
Copy = mybir.ActivationFunctionType.Copy

import os

# M=1 column-tiled softmax-denominator matmuls + Pool partition_broadcast
# normalize; when off, falls back to full ones[128,128] matmuls + DVE
# reciprocal like the original kernel.
COLTILE_L = os.environ.get("K_COLTILE_L", "1") == "1"
DMA_ROPE = os.environ.get("K_DMA_ROPE", "1") == "1"
# broadcast engine for 1/l: "pool" = gpsimd partition_broadcast,
# "pe" = rank-1 matmul (ones column x l row) into PSUM + DVE drain.
BCAST = os.environ.get("K_BCAST", "pe")


def _build(nc):
    xp = nc.dram_tensor("xp", [NQ, 128, KC, 512], BF16, kind="ExternalInput").ap()
    wqkv = nc.dram_tensor("wqkv", [128, KC, 4, 128], BF16, kind="ExternalInput").ap()
    wo = nc.dram_tensor("wo", [128, HL, D], BF16, kind="ExternalInput").ap()
    cos2 = nc.dram_tensor("cos2", [128, T], BF16, kind="ExternalInput").ap()
    sinsig = nc.dram_tensor("sinsig", [128, T], BF16, kind="ExternalInput").ap()
    y = nc.dram_tensor("y", [T, D], BF16, kind="ExternalOutput").ap()

    with tile.TileContext(nc) as tc, ExitStack() as ctx:
        const = ctx.enter_context(tc.tile_pool(name="const", bufs=1))
        xpool = ctx.enter_context(tc.tile_pool(name="xp", bufs=2))
        psum = ctx.enter_context(tc.tile_pool(name="ps", bufs=4, space="PSUM"))
        opsum = ctx.enter_context(tc.tile_pool(name="ops", bufs=2, space="PSUM"))
        laux = ctx.enter_context(tc.tile_pool(name="laux", bufs=2, space="PSUM"))
        ppool = ctx.enter_context(tc.tile_pool(name="pt", bufs=6))
        swpool = ctx.enter_context(tc.tile_pool(name="sw", bufs=6))
        lspool = ctx.enter_context(tc.tile_pool(name="ls", bufs=2))
        lbpool = ctx.enter_context(tc.tile_pool(name="lb", bufs=2))
        yrow = ctx.enter_context(tc.tile_pool(name="yr", bufs=2))

        wqkv_sb = const.tile([128, KC, 4, 128], BF16, tag="wqkv")
        wo_sb = const.tile([128, HL, D], BF16, tag="wo")
        cos_sb = const.tile([128, T], BF16, tag="cos")
        sin_sb = const.tile([128, T], BF16, tag="sin")
        ident = const.tile([128, 128], BF16, tag="ident")
        qkvT = const.tile([128, 4, T], BF16, tag="qkvT")   # Q0,Q1,K,V as [d,t]
        vnat = const.tile([128, NJ, 128], BF16, tag="vnat")  # V natural [j, d]
        ones_sb = const.tile([128, 128], BF16, tag="ones")
        oT = const.tile([128, HL, T], BF16, tag="oT")

        make_identity(nc, ident[:])
        nc.vector.memset(ones_sb[:], 1.0)
        if not DMA_ROPE:
            # rotate-half permutation: perm[:, j] = ident[:, (j+64)%128]
            perm_sb = const.tile([128, 128], BF16, tag="perm")
            nc.vector.tensor_copy(perm_sb[:, 0:64], ident[:, 64:128])
            nc.vector.tensor_copy(perm_sb[:, 64:128], ident[:, 0:64])

        def emit_oproj(oq):
            # partial o_proj y = O^T.T @ Wo for q-tile oq's 4 row blocks.
            # h2-outer so each oT row block is loaded as weights only twice;
            # PSUM->SBUF casts alternate DVE/ACT to avoid a single-engine
            # drain (ACT's exp backlog builds later in the window).
            for tsub in range(4):
                ti = oq * 4 + tsub
                yr = yrow.tile([128, D], BF16, tag="yr", name=f"yr{ti}")
                yps = [
                    psum.tile([128, 512], F32, tag="ps", name=f"yp{ti}_{mi}")
                    for mi in range(NM)
                ]
                for h2 in range(HL):
                    for mi in range(NM):
                        nc.tensor.matmul(
                            yps[mi][:],
                            lhsT=oT[:, h2, bass.ts(ti, 128)],
                            rhs=wo_sb[:, h2, bass.ts(mi, 512)],
                            start=(h2 == 0),
                            stop=(h2 == HL - 1),
                        )
                for mi in range(NM):
                    if mi % 2 == 0:
                        nc.vector.tensor_copy(yr[:, bass.ts(mi, 512)], yps[mi][:])
                    else:
                        nc.scalar.copy(yr[:, bass.ts(mi, 512)], yps[mi][:])
                nc.sync.dma_start(y[bass.ts(ti, 128), :], yr[:])

        kT = qkvT[:, 2, :]
        xts = {}

        def issue_xt(n):
            # prefetch window n's x tile; called one window ahead so the
            # ~20us transfer overlaps the previous window's attention.
            xt = xpool.tile([128, KC, 512], BF16, tag="xt", name=f"xt{n}")
            nch = 8 if n == 0 else 4
            step = KC // nch
            for kq in range(nch):
                nc.gpsimd.dma_start(
                    xt[:, bass.ts(kq, step), :], xp[n, :, bass.ts(kq, step), :]
                )
                if n == 0:  # interleave weight chunks with the first x tile
                    nc.gpsimd.dma_start(
                        wqkv_sb[:, bass.ts(kq, 2)], wqkv[:, bass.ts(kq, 2)]
                    )
            xts[n] = xt

        for n in range(NQ):
            ns = bass.ts(n, 512)
            if n == 0:
                nc.sync.dma_start(cos_sb[:], cos2[:])
                nc.sync.dma_start(sin_sb[:], sinsig[:])
                issue_xt(0)
                nc.sync.dma_start(wo_sb[:], wo[:])
            xt = xts.pop(n)

            # fused q/k/v projection into raw (pre-RoPE) tiles, [d, t];
            # k-outer so each arriving x chunk immediately feeds 4 matmuls.
            qraw = swpool.tile([128, 3, 512], BF16, tag="qraw", name=f"qraw{n}")

            def proj_drain(m, prjm):
                dst = qkvT[:, m, ns] if m == 3 else qraw[:, m, :]
                if m % 2 == 0:
                    nc.vector.tensor_copy(dst, prjm[:])
                else:
                    nc.scalar.copy(dst, prjm[:])

            def emit_rope(i):
                # RoPE rotate-half: partition swap via two SBUF->SBUF DMAs
                # (no PE permutation matmul, no ACT drain). qraw is read-only
                # here so the DVE chain is off the DMA's WAR path.
                src = qraw[:, i, :]
                dst = qkvT[:, i, ns]
                swp = swpool.tile([128, 512], BF16, tag="sw", name=f"sw{n}_{i}")
                if DMA_ROPE:
                    nc.sync.dma_start(swp[0:64, :], qraw[64:128, i, :])
                    nc.sync.dma_start(swp[64:128, :], qraw[0:64, i, :])
                else:
                    sw_ps = psum.tile([128, 512], F32, tag="ps", name=f"swp{n}_{i}")
                    nc.tensor.matmul(
                        sw_ps[:], lhsT=perm_sb[:], rhs=src, start=True, stop=True
                    )
                    nc.scalar.copy(swp[:], sw_ps[:])
                nc.vector.tensor_mul(dst, src, cos_sb[:, ns])
                nc.vector.tensor_mul(swp[:], swp[:], sin_sb[:, ns])
                nc.vector.tensor_add(dst, dst, swp[:])

            if n == 0:
                # k-outer: each arriving x chunk immediately feeds 4 matmuls
                # (startup is DMA-bound); all four drains land at the end.
                prj = [
                    psum.tile([128, 512], F32, tag="ps", name=f"prj{n}_{m}")
                    for m in range(4)
                ]
                for k in range(KC):
                    for m in range(4):
                        nc.tensor.matmul(
                            prj[m][:],
                            lhsT=wqkv_sb[:, k, m, :],
                            rhs=xt[:, k, :],
                            start=(k == 0),
                            stop=(k == KC - 1),
                        )
                issue_xt(n + 1)
                for m in range(4):
                    proj_drain(m, prj[m])
                for i in (0, 1, 2):
                    emit_rope(i)
            else:
                # m-outer: prj[m] completes a quarter of the way in, so its
                # drain + RoPE chain overlaps the rest of the projection and
                # the attention matmuls never wait on the rope output.
                for m in range(4):
                    prjm = psum.tile([128, 512], F32, tag="ps", name=f"prj{n}_{m}")
                    for k in range(KC):
                        nc.tensor.matmul(
                            prjm[:],
                            lhsT=wqkv_sb[:, k, m, :],
                            rhs=xt[:, k, :],
                            start=(k == 0),
                            stop=(k == KC - 1),
                        )
                    if m == 0 and n + 1 < NQ:
                        issue_xt(n + 1)
                    proj_drain(m, prjm)
                    if m < 3:
                        emit_rope(m)

            # o_proj for the previous q-tile: placed here so the PE has
            # dense work while the rope DMAs + DVE chain complete.
            if n > 0:
                emit_oproj(n - 1)

            # V^T -> V natural [j, d] for this window's 4 kv tiles
            for jt in range(4 * n, 4 * n + 4):
                tp = psum.tile([128, 128], BF16, tag="ps", name=f"vtp{jt}")
                nc.tensor.transpose(tp[:], qkvT[:, 3, bass.ts(jt, 128)], ident[:])
                nc.vector.tensor_copy(vnat[:, jt, :], tp[:])

            # causal attention for q-tile qi=n, both heads interleaved per
            # kv tile (they share K/V). Diagonal blocks are column-trimmed:
            # block jt touches only q columns >= 128*kd (kd = jt-4qi), so
            # S / exp / O / l all run on the [s0:512] subrange and the mask
            # only needs the first 128 columns of that subrange.
            qi = n
            njt = 4 * (qi + 1)
            ot = [
                opsum.tile([128, 512], F32, tag="oacc", name=f"oacc{qi}_{h}")
                for h in range(HL)
            ]
            # softmax denominators: head h accumulates an M=1 row at
            # partition 32*h via column-tiled matmuls (PE col groups 0 and
            # 32 run them concurrently); separate banks per head so each
            # accumulation group owns its bank.
            lb = [
                laux.tile([128, 512], F32, tag="laux", name=f"lacc{qi}_{h}")
                for h in range(HL)
            ]
            # Software-pipelined by one kv block: O/l accumulation for block
            # jt is emitted after the S matmuls of block jt+1, so the PE has
            # S streams to chew on while ACT exp + Pool mask of block jt
            # complete.
            pend = None  # (jt, pts) awaiting O/l emission

            def emit_s(jt):
                kd = jt - 4 * qi
                s0 = 128 * kd if kd > 0 else 0
                w = 512 - s0
                pts = []
                for h in range(HL):
                    sps = psum.tile(
                        [128, 512], F32, tag="ps", name=f"sps{qi}_{jt}_{h}"
                    )
                    nc.tensor.matmul(
                        sps[:, :w],
                        lhsT=kT[:, bass.ts(jt, 128)],
                        rhs=qkvT[:, h, qi * 512 + s0 : (qi + 1) * 512],
                        start=True,
                        stop=True,
                    )
                    pt = ppool.tile([128, 512], BF16, tag="pt", name=f"pt{jt}_{h}")
                    nc.scalar.activation(pt[:, :w], sps[:, :w], Exp, scale=SCALE)
                    if kd >= 0:  # mask the 128-col block straddling the diagonal
                        nc.gpsimd.affine_select(
                            out=pt[:, :128],
                            in_=pt[:, :128],
                            compare_op=mybir.AluOpType.is_ge,
                            fill=0.0,
                            base=0,
                            channel_multiplier=-1,
                            pattern=[[1, 128]],
                        )
                    pts.append((pt, s0, w))
                return pts

            def emit_ol(jt, pts):
                for h in range(HL):
                    pt, ps0, pw = pts[h]
                    nc.tensor.matmul(
                        ot[h][:, ps0:],
                        lhsT=vnat[:, jt, :],
                        rhs=pt[:, :pw],
                        start=(jt == 0),
                        stop=(jt == njt - 1),
                    )
                for h in range(HL):
                    pt, ps0, pw = pts[h]
                    if COLTILE_L:
                        nc.tensor.matmul(
                            lb[h][32 * h : 32 * h + 1, ps0:],
                            lhsT=ones_sb[:, :1],
                            rhs=pt[:, :pw],
                            start=(jt == 0),
                            stop=(jt == njt - 1),
                        )
                    else:
                        nc.tensor.matmul(
                            lb[h][:, ps0:],
                            lhsT=ones_sb[:],
                            rhs=pt[:, :pw],
                            start=(jt == 0),
                            stop=(jt == njt - 1),
                        )

            for jt in range(njt):
                pts = emit_s(jt)
                if pend is not None:
                    emit_ol(*pend)
                pend = (jt, pts)
            emit_ol(*pend)

            if COLTILE_L:
                # normalize: 1/l broadcast across partitions, O^T scaled on
                # DVE
                lsb = lspool.tile([128, 512], F32, tag="ls", name=f"ls{qi}")
                for h in range(HL):
                    p = 32 * h
                    nc.vector.tensor_copy(lsb[p : p + 1, :], lb[h][p : p + 1, :])
                    nc.vector.reciprocal_approx_fast(
                        lsb[p : p + 1, :], lsb[p : p + 1, :]
                    )
                    lbc = lbpool.tile([128, 512], F32, tag="lb", name=f"lbc{qi}_{h}")
                    if BCAST == "pool":
                        nc.gpsimd.partition_broadcast(lbc[:], lsb[p : p + 1, :])
                        nc.vector.tensor_mul(oT[:, h, ns], ot[h][:], lbc[:])
                    else:
                        # rank-1 matmul broadcast: out[i, q] = 1 * l^-1[q];
                        # reuses the head's l bank after its row was copied
                        # out.
                        bps = laux.tile(
                            [128, 512], F32, tag="laux", name=f"bps{qi}_{h}"
                        )
                        nc.tensor.matmul(
                            bps[:],
                            lhsT=ones_sb[p : p + 1, :],
                            rhs=lsb[p : p + 1, :],
                            start=True,
                            stop=True,
                        )
                        nc.vector.tensor_copy(lbc[:], bps[:])
                        nc.vector.tensor_mul(oT[:, h, ns], ot[h][:], lbc[:])
            else:
                for h in range(HL):
                    lbc = lbpool.tile([128, 512], F32, tag="lb", name=f"lbc{qi}_{h}")
                    nc.vector.reciprocal_approx_fast(lbc[:], lb[h][:])
                    nc.vector.tensor_mul(oT[:, h, ns], ot[h][:], lbc[:])
        emit_oproj(NQ - 1)


_CACHE = {}


def _get_program():
    if "nc" not in _CACHE:
        nc = bacc.Bacc(
            "TRN2", target_bir_lowering=False, debug=False, num_devices=N_CORES
        )
        _build(nc)
        nc.compile()
        _CACHE["nc"] = nc
    return _CACHE["nc"]


def _rope_tables():
    inv_freq = 1.0 / (ROPE_BASE ** (np.arange(64, dtype=np.float64) / 64))
    ang = np.arange(T, dtype=np.float64)[:, None] * inv_freq[None, :]  # [T, 64]
    cos = np.cos(ang).T  # [64, T]
    sin = np.sin(ang).T
    cos2 = np.concatenate([cos, cos], axis=0).astype(NPBF16)
    sinsig = np.concatenate([-sin, sin], axis=0).astype(NPBF16)
    return cos2, sinsig


def kernel(x, Wq, Wk, Wv, Wo):
    x = np.asarray(x, dtype=np.float32)
    Wq = np.asarray(Wq, dtype=np.float32)
    Wk = np.asarray(Wk, dtype=np.float32)
    Wv = np.asarray(Wv, dtype=np.float32)
    Wo = np.asarray(Wo, dtype=np.float32)

    # x[t, c] -> xp[n, p, k, j] = x[n*512+j, k*128+p]; contiguous per partition.
    xp = np.ascontiguousarray(
        x.reshape(T, D).reshape(NQ, 512, KC, 128).transpose(0, 3, 2, 1)
    ).astype(NPBF16)
    cos2, sinsig = _rope_tables()

    in_maps = []
    for c in range(N_CORES):
        h0, h1 = 2 * c, 2 * c + 1
        kv = c // 2
        wqkv_c = np.concatenate(
            [
                Wq[:, h0 * DH:(h0 + 1) * DH],
                Wq[:, h1 * DH:(h1 + 1) * DH],
                Wk[:, kv * DH:(kv + 1) * DH],
                Wv[:, kv * DH:(kv + 1) * DH],
            ],
            axis=1,
        )  # [D, 512]
        wqkv_pre = np.ascontiguousarray(
            wqkv_c.reshape(KC, 128, 4, 128).transpose(1, 0, 2, 3)
        ).astype(NPBF16)
        wo_pre = np.ascontiguousarray(
            np.stack(
                [Wo[h0 * DH:(h0 + 1) * DH, :], Wo[h1 * DH:(h1 + 1) * DH, :]], axis=0
            ).transpose(1, 0, 2)
        ).astype(NPBF16)
        in_maps.append(
            {
                "xp": xp,
                "wqkv": wqkv_pre,
                "wo": wo_pre,
                "cos2": cos2,
                "sinsig": sinsig,
            }
        )

    nc = _get_program()
    res = run_bass_kernel_spmd(nc, in_maps, list(range(N_CORES)))
    out = np.zeros((T, D), dtype=np.float32)
    for c in range(N_CORES):
        out += res.results[c]["y"].astype(np.float32)
    return out.reshape(B, T, D)


# revision 29
# speedup vs baseline: 1.1126x; 1.0842x over previous
"""Trainium2 Bass kernel for causal GQA attention (B=1, T=4096, D=2048,
H=16, Hkv=4, Dh=128, RoPE) sharded over 8 NeuronCores.

Sharding: tensor-parallel over heads — each core owns 2 q-heads and the
kv head they share (core c: q-heads {2c, 2c+1}, kv head c//2). Each core
computes its q/k/v projections, RoPE, causal attention and its partial
o_proj contribution y_c = O_c @ Wo_c; the host sums the 8 partials.

On-device dataflow (bf16 into the PE, f32 accumulation):
  xT tiles -> Q^T/K^T/V^T [d,t] (k-outer proj, DVE/ACT drains) ->
  RoPE rotate-half via SBUF-SBUF DMA partition swap + DVE muls ->
  S^T = K^T.T@Q^T per (jt 128, q 512) with causal column trim on
  diagonal blocks -> exp on ACT (PSUM->SBUF bf16, fused 1/sqrt(dh))
  -> diagonal mask on Pool (affine_select, 128-col region only) ->
  O^T[d,q] += V.T @ P^T (PE) and l[q] += ones1.T @ P^T as M=1
  column-tiled matmuls (both heads concurrently in PE col groups
  0 and 32) -> l: copy+recip (DVE), partition broadcast (Pool),
  O^T normalize (DVE) -> y = O^T.T @ Wo, drains DVE/ACT alternated.
"""

import sys

sys.path.insert(0, "/opt/trn_rl_repo")

import math
from contextlib import ExitStack

import ml_dtypes
import numpy as np

import concourse.bass as bass
import concourse.tile as tile
from concourse import bacc, mybir
from concourse.bass_utils import run_bass_kernel_spmd
from concourse.masks import make_identity

BF16 = mybir.dt.bfloat16
F32 = mybir.dt.float32
NPBF16 = ml_dtypes.bfloat16

B, T, D = 1, 4096, 2048
H, HKV, DH = 16, 4, 128
GROUP = H // HKV
ROPE_BASE = 10000.0
N_CORES = 8
HL = H // N_CORES  # q-heads per core
KC = D // 128      # contraction tiles for projections
NQ = T // 512      # 512-wide q tiles
NJ = T // 128      # 128-wide kv tiles
NT = T // 128      # 128-row output tiles
NM = D // 512      # 512-wide output column tiles
SCALE = 1.0 / math.sqrt(DH)

Exp = mybir.ActivationFunctionType.Exp
Copy = mybir.ActivationFunctionType.Copy

import os

# M=1 column-tiled softmax-denominator matmuls + Pool partition_broadcast
# normalize; when off, falls back to full ones[128,128] matmuls + DVE
# reciprocal like the original kernel.
COLTILE_L = os.environ.get("K_COLTILE_L", "0") == "1"
DMA_ROPE = os.environ.get("K_DMA_ROPE", "1") == "1"
# broadcast engine for 1/l: "pool" = gpsimd partition_broadcast,
# "pe" = rank-1 matmul (ones column x l row) into PSUM + DVE drain.
BCAST = os.environ.get("K_BCAST", "pe")


def _build(nc):
    xp = nc.dram_tensor("xp", [NQ, 128, KC, 512], BF16, kind="ExternalInput").ap()
    wqkv = nc.dram_tensor("wqkv", [128, KC, 4, 128], BF16, kind="ExternalInput").ap()
    wo = nc.dram_tensor("wo", [128, HL, D], BF16, kind="ExternalInput").ap()
    cos2 = nc.dram_tensor("cos2", [128, T], BF16, kind="ExternalInput").ap()
    sinsig = nc.dram_tensor("sinsig", [128, T], BF16, kind="ExternalInput").ap()
    y = nc.dram_tensor("y", [T, D], BF16, kind="ExternalOutput").ap()

    with tile.TileContext(nc) as tc, ExitStack() as ctx:
        const = ctx.enter_context(tc.tile_pool(name="const", bufs=1))
        xpool = ctx.enter_context(tc.tile_pool(name="xp", bufs=2))
        psum = ctx.enter_context(tc.tile_pool(name="ps", bufs=4, space="PSUM"))
        opsum = ctx.enter_context(tc.tile_pool(name="ops", bufs=2, space="PSUM"))
        laux = ctx.enter_context(tc.tile_pool(name="laux", bufs=2, space="PSUM"))
        ppool = ctx.enter_context(tc.tile_pool(name="pt", bufs=6))
        swpool = ctx.enter_context(tc.tile_pool(name="sw", bufs=6))
        lspool = ctx.enter_context(tc.tile_pool(name="ls", bufs=2))
        lbpool = ctx.enter_context(tc.tile_pool(name="lb", bufs=2))
        yrow = ctx.enter_context(tc.tile_pool(name="yr", bufs=4))

        wqkv_sb = const.tile([128, KC, 4, 128], BF16, tag="wqkv")
        wo_sb = const.tile([128, HL, D], BF16, tag="wo")
        cos_sb = const.tile([128, T], BF16, tag="cos")
        sin_sb = const.tile([128, T], BF16, tag="sin")
        ident = const.tile([128, 128], BF16, tag="ident")
        qkvT = const.tile([128, 4, T], BF16, tag="qkvT")   # Q0,Q1,K,V as [d,t]
        vnat = const.tile([128, NJ, 128], BF16, tag="vnat")  # V natural [j, d]
        ones_sb = const.tile([128, 128], BF16, tag="ones")
        oT = const.tile([128, HL, T], BF16, tag="oT")

        make_identity(nc, ident[:])
        nc.vector.memset(ones_sb[:], 1.0)
        if not DMA_ROPE:
            # rotate-half permutation: perm[:, j] = ident[:, (j+64)%128]
            perm_sb = const.tile([128, 128], BF16, tag="perm")
            nc.vector.tensor_copy(perm_sb[:, 0:64], ident[:, 64:128])
            nc.vector.tensor_copy(perm_sb[:, 64:128], ident[:, 0:64])

        def emit_oproj(oq):
            # partial o_proj y = O^T.T @ Wo for q-tile oq's 4 row blocks.
            # h2-outer so each oT row block is loaded as weights only twice;
            # PSUM->SBUF casts alternate DVE/ACT to avoid a single-engine
            # drain (ACT's exp backlog builds later in the window).
            for tsub in range(4):
                ti = oq * 4 + tsub
                yr = yrow.tile([128, D], BF16, tag="yr", name=f"yr{ti}")
                yps = [
                    psum.tile([128, 512], F32, tag="ps", name=f"yp{ti}_{mi}")
                    for mi in range(NM)
                ]
                for h2 in range(HL):
                    for mi in range(NM):
                        nc.tensor.matmul(
                            yps[mi][:],
                            lhsT=oT[:, h2, bass.ts(ti, 128)],
                            rhs=wo_sb[:, h2, bass.ts(mi, 512)],
                            start=(h2 == 0),
                            stop=(h2 == HL - 1),
                        )
                for mi in range(NM):
                    # mid-kernel o_proj drains stay off ACT so the next
                    # window's exps aren't queued behind them; the final
                    # window has no exps left, so alternate to finish faster.
                    if oq == NQ - 1 and mi % 2 == 1:
                        nc.scalar.copy(yr[:, bass.ts(mi, 512)], yps[mi][:])
                    else:
                        nc.vector.tensor_copy(yr[:, bass.ts(mi, 512)], yps[mi][:])
                nc.sync.dma_start(y[bass.ts(ti, 128), :], yr[:])

        kT = qkvT[:, 2, :]
        xts = {}

        def issue_xt(n):
            # prefetch window n's x tile; called one window ahead so the
            # ~20us transfer overlaps the previous window's attention.
            xt = xpool.tile([128, KC, 512], BF16, tag="xt", name=f"xt{n}")
            nch = 8 if n == 0 else 4
            step = KC // nch
            for kq in range(nch):
                nc.gpsimd.dma_start(
                    xt[:, bass.ts(kq, step), :], xp[n, :, bass.ts(kq, step), :]
                )
                if n == 0:  # interleave weight chunks with the first x tile
                    nc.gpsimd.dma_start(
                        wqkv_sb[:, bass.ts(kq, 2)], wqkv[:, bass.ts(kq, 2)]
                    )
            xts[n] = xt

        for n in range(NQ):
            ns = bass.ts(n, 512)
            if n == 0:
                nc.sync.dma_start(cos_sb[:], cos2[:])
                nc.sync.dma_start(sin_sb[:], sinsig[:])
                issue_xt(0)
                nc.sync.dma_start(wo_sb[:], wo[:])
            xt = xts.pop(n)

            # fused q/k/v projection into raw (pre-RoPE) tiles, [d, t];
            # k-outer so each arriving x chunk immediately feeds 4 matmuls.
            qraw = swpool.tile([128, 3, 512], BF16, tag="qraw", name=f"qraw{n}")

            def proj_drain(m, prjm):
                dst = qkvT[:, m, ns] if m == 3 else qraw[:, m, :]
                if m % 2 == 0:
                    nc.vector.tensor_copy(dst, prjm[:])
                else:
                    nc.scalar.copy(dst, prjm[:])

            def emit_rope(i):
                # RoPE rotate-half: partition swap via two SBUF->SBUF DMAs
                # (no PE permutation matmul, no ACT drain). qraw is read-only
                # here so the DVE chain is off the DMA's WAR path.
                src = qraw[:, i, :]
                dst = qkvT[:, i, ns]
                swp = swpool.tile([128, 512], BF16, tag="sw", name=f"sw{n}_{i}")
                if DMA_ROPE:
                    nc.sync.dma_start(swp[0:64, :], qraw[64:128, i, :])
                    nc.sync.dma_start(swp[64:128, :], qraw[0:64, i, :])
                else:
                    sw_ps = psum.tile([128, 512], F32, tag="ps", name=f"swp{n}_{i}")
                    nc.tensor.matmul(
                        sw_ps[:], lhsT=perm_sb[:], rhs=src, start=True, stop=True
                    )
                    nc.scalar.copy(swp[:], sw_ps[:])
                nc.vector.tensor_mul(dst, src, cos_sb[:, ns])
                nc.vector.tensor_mul(swp[:], swp[:], sin_sb[:, ns])
                nc.vector.tensor_add(dst, dst, swp[:])

            if n == 0:
                # k-outer: each arriving x chunk immediately feeds 4 matmuls
                # (startup is DMA-bound); all four drains land at the end.
                prj = [
                    psum.tile([128, 512], F32, tag="ps", name=f"prj{n}_{m}")
                    for m in range(4)
                ]
                for k in range(KC):
                    for m in range(4):
                        nc.tensor.matmul(
                            prj[m][:],
                            lhsT=wqkv_sb[:, k, m, :],
                            rhs=xt[:, k, :],
                            start=(k == 0),
                            stop=(k == KC - 1),
                        )
                issue_xt(n + 1)
                for m in range(4):
                    proj_drain(m, prj[m])
                for i in (0, 1, 2):
                    emit_rope(i)
            else:
                # m-outer: prj[m] completes a quarter of the way in, so its
                # drain + RoPE chain overlaps the rest of the projection and
                # the attention matmuls never wait on the rope output.
                for m in range(4):
                    prjm = psum.tile([128, 512], F32, tag="ps", name=f"prj{n}_{m}")
                    for k in range(KC):
                        nc.tensor.matmul(
                            prjm[:],
                            lhsT=wqkv_sb[:, k, m, :],
                            rhs=xt[:, k, :],
                            start=(k == 0),
                            stop=(k == KC - 1),
                        )
                    if m == 0 and n + 1 < NQ:
                        issue_xt(n + 1)
                    proj_drain(m, prjm)
                    if m < 3:
                        emit_rope(m)

            # o_proj for the previous q-tile: placed here so the PE has
            # dense work while the rope DMAs + DVE chain complete.
            if n > 0:
                emit_oproj(n - 1)

            # V^T -> V natural [j, d] for this window's 4 kv tiles
            for jt in range(4 * n, 4 * n + 4):
                tp = psum.tile([128, 128], BF16, tag="ps", name=f"vtp{jt}")
                nc.tensor.transpose(tp[:], qkvT[:, 3, bass.ts(jt, 128)], ident[:])
                nc.vector.tensor_copy(vnat[:, jt, :], tp[:])

            # causal attention for q-tile qi=n, both heads interleaved per
            # kv tile (they share K/V). Diagonal blocks are column-trimmed:
            # block jt touches only q columns >= 128*kd (kd = jt-4qi), so
            # S / exp / O / l all run on the [s0:512] subrange and the mask
            # only needs the first 128 columns of that subrange.
            qi = n
            njt = 4 * (qi + 1)
            ot = [
                opsum.tile([128, 512], F32, tag="oacc", name=f"oacc{qi}_{h}")
                for h in range(HL)
            ]
            # softmax denominators: head h accumulates an M=1 row at
            # partition 32*h via column-tiled matmuls (PE col groups 0 and
            # 32 run them concurrently); separate banks per head so each
            # accumulation group owns its bank.
            lb = [
                laux.tile([128, 512], F32, tag="laux", name=f"lacc{qi}_{h}")
                for h in range(HL)
            ]
            # Software-pipelined by one kv block: O/l accumulation for block
            # jt is emitted after the S matmuls of block jt+1, so the PE has
            # S streams to chew on while ACT exp + Pool mask of block jt
            # complete.
            pend = None  # (jt, pts) awaiting O/l emission

            def emit_s(jt):
                kd = jt - 4 * qi
                s0 = 128 * kd if kd > 0 else 0
                w = 512 - s0
                pts = []
                for h in range(HL):
                    sps = psum.tile(
                        [128, 512], F32, tag="ps", name=f"sps{qi}_{jt}_{h}"
                    )
                    nc.tensor.matmul(
                        sps[:, :w],
                        lhsT=kT[:, bass.ts(jt, 128)],
                        rhs=qkvT[:, h, qi * 512 + s0 : (qi + 1) * 512],
                        start=True,
                        stop=True,
                    )
                    pt = ppool.tile([128, 512], BF16, tag="pt", name=f"pt{jt}_{h}")
                    nc.scalar.activation(pt[:, :w], sps[:, :w], Exp, scale=SCALE)
                    if kd >= 0:  # mask the 128-col block straddling the diagonal
                        nc.gpsimd.affine_select(
                            out=pt[:, :128],
                            in_=pt[:, :128],
                            compare_op=mybir.AluOpType.is_ge,
                            fill=0.0,
                            base=0,
                            channel_multiplier=-1,
                            pattern=[[1, 128]],
                        )
                    pts.append((pt, s0, w))
                return pts

            def emit_ol(jt, pts):
                for h in range(HL):
                    pt, ps0, pw = pts[h]
                    nc.tensor.matmul(
                        ot[h][:, ps0:],
                        lhsT=vnat[:, jt, :],
                        rhs=pt[:, :pw],
                        start=(jt == 0),
                        stop=(jt == njt - 1),
                    )
                for h in range(HL):
                    pt, ps0, pw = pts[h]
                    if COLTILE_L:
                        nc.tensor.matmul(
                            lb[h][32 * h : 32 * h + 1, ps0:],
                            lhsT=ones_sb[:, :1],
                            rhs=pt[:, :pw],
                            start=(jt == 0),
                            stop=(jt == njt - 1),
                        )
                    else:
                        nc.tensor.matmul(
                            lb[h][:, ps0:],
                            lhsT=ones_sb[:],
                            rhs=pt[:, :pw],
                            start=(jt == 0),
                            stop=(jt == njt - 1),
                        )

            for jt in range(njt):
                pts = emit_s(jt)
                if pend is not None:
                    emit_ol(*pend)
                pend = (jt, pts)
            emit_ol(*pend)

            if COLTILE_L:
                # normalize: 1/l broadcast across partitions, O^T scaled on
                # DVE
                lsb = lspool.tile([128, 512], F32, tag="ls", name=f"ls{qi}")
                for h in range(HL):
                    p = 32 * h
                    nc.vector.tensor_copy(lsb[p : p + 1, :], lb[h][p : p + 1, :])
                    nc.vector.reciprocal_approx_fast(
                        lsb[p : p + 1, :], lsb[p : p + 1, :]
                    )
                    lbc = lbpool.tile([128, 512], F32, tag="lb", name=f"lbc{qi}_{h}")
                    if BCAST == "pool":
                        nc.gpsimd.partition_broadcast(lbc[:], lsb[p : p + 1, :])
                        nc.vector.tensor_mul(oT[:, h, ns], ot[h][:], lbc[:])
                    else:
                        # rank-1 matmul broadcast: out[i, q] = 1 * l^-1[q];
                        # reuses the head's l bank after its row was copied
                        # out.
                        bps = laux.tile(
                            [128, 512], F32, tag="laux", name=f"bps{qi}_{h}"
                        )
                        # NOTE: col-tiled l is off by default — it produced
                        # NaN on hardware despite passing CoreSim.
                        nc.tensor.matmul(
                            bps[:],
                            lhsT=ones_sb[p : p + 1, :],
                            rhs=lsb[p : p + 1, :],
                            start=True,
                            stop=True,
                        )
                        nc.vector.tensor_copy(lbc[:], bps[:])
                        nc.vector.tensor_mul(oT[:, h, ns], ot[h][:], lbc[:])
            else:
                for h in range(HL):
                    lbc = lbpool.tile([128, 512], F32, tag="lb", name=f"lbc{qi}_{h}")
                    nc.vector.reciprocal_approx_fast(lbc[:], lb[h][:])
                    nc.vector.tensor_mul(oT[:, h, ns], ot[h][:], lbc[:])
        emit_oproj(NQ - 1)


_CACHE = {}


def _get_program():
    if "nc" not in _CACHE:
        nc = bacc.Bacc(
            "TRN2", target_bir_lowering=False, debug=False, num_devices=N_CORES
        )
        _build(nc)
        nc.compile()
        _CACHE["nc"] = nc
    return _CACHE["nc"]


def _rope_tables():
    inv_freq = 1.0 / (ROPE_BASE ** (np.arange(64, dtype=np.float64) / 64))
    ang = np.arange(T, dtype=np.float64)[:, None] * inv_freq[None, :]  # [T, 64]
    cos = np.cos(ang).T  # [64, T]
    sin = np.sin(ang).T
    cos2 = np.concatenate([cos, cos], axis=0).astype(NPBF16)
    sinsig = np.concatenate([-sin, sin], axis=0).astype(NPBF16)
    return cos2, sinsig


def kernel(x, Wq, Wk, Wv, Wo):
    x = np.asarray(x, dtype=np.float32)
    Wq = np.asarray(Wq, dtype=np.float32)
    Wk = np.asarray(Wk, dtype=np.float32)
    Wv = np.asarray(Wv, dtype=np.float32)
    Wo = np.asarray(Wo, dtype=np.float32)

    # x[t, c] -> xp[n, p, k, j] = x[n*512+j, k*128+p]; contiguous per partition.
    xp = np.ascontiguousarray(
        x.reshape(T, D).reshape(NQ, 512, KC, 128).transpose(0, 3, 2, 1)
    ).astype(NPBF16)
    cos2, sinsig = _rope_tables()

    in_maps = []
    for c in range(N_CORES):
        h0, h1 = 2 * c, 2 * c + 1
        kv = c // 2
        wqkv_c = np.concatenate(
            [
                Wq[:, h0 * DH:(h0 + 1) * DH],
                Wq[:, h1 * DH:(h1 + 1) * DH],
                Wk[:, kv * DH:(kv + 1) * DH],
                Wv[:, kv * DH:(kv + 1) * DH],
            ],
            axis=1,
        )  # [D, 512]
        wqkv_pre = np.ascontiguousarray(
            wqkv_c.reshape(KC, 128, 4, 128).transpose(1, 0, 2, 3)
        ).astype(NPBF16)
        wo_pre = np.ascontiguousarray(
            np.stack(
                [Wo[h0 * DH:(h0 + 1) * DH, :], Wo[h1 * DH:(h1 + 1) * DH, :]], axis=0
            ).transpose(1, 0, 2)
        ).astype(NPBF16)
        in_maps.append(
            {
                "xp": xp,
                "wqkv": wqkv_pre,
                "wo": wo_pre,
                "cos2": cos2,
                "sinsig": sinsig,
            }
        )

    nc = _get_program()
    res = run_bass_kernel_spmd(nc, in_maps, list(range(N_CORES)))
    out = np.zeros((T, D), dtype=np.float32)
    for c in range(N_CORES):
        out += res.results[c]["y"].astype(np.float32)
    return out.reshape(B, T, D)
